# revision 28
# baseline (speedup 1.0000x reference)
"""Trainium2 Bass kernel for the ActorMCP mixture-of-experts policy network.

Data-parallel over 8 NeuronCores: batch 32768 -> 4096 rows/core, weights
replicated. All activations live transposed [feature(partitions), batch(free)]
so every layer is out.T = W.T @ x.T with W in its natural [fan_in, fan_out]
layout as the stationary matmul operand.

Engine budget choices (from trace analysis):
- Scalar engine runs ONLY Exp/Tanh (one activation-table set) for the
  whole batch loop; the single Sqrt runs once at the end over all batch
  columns (forced there by its data dependency), so there are exactly
  2 ACT_TABLE_LOADs in the kernel.
- sigmoid(z) = 0.5*tanh(0.5 z) + 0.5 (tanh shares the exp table set).
- ELU = max(x+b, min(exp(x+b)-1, 0)) as ONE fused custom DVE op after the
  ACT exp; w*clip(inv_var) is a second custom op. Custom DVE uop tables
  are registered at runtime and baked into the NEFF.
- GpSimd does no element-wise work (measured 7.5us/op + DVE port stalls);
  it only issues the weight DMAs so they ride a different queue than the
  per-tile input loads on Sync.
- Weights/inputs host-packed k-tile-interleaved [128, K/128*M] so big
  contiguous DMAs feed SBUF; first-consumed slabs split per k-chunk so
  the first matmul starts as early as possible.
- Software pipelining: tile n's per-expert GEMMs + mixture epilogue are
  emitted inside tile n+1's backbone at points chosen so the PE FIFO
  always holds ready work across the narrow layer-transition funnels,
  and the funnel-critical exps sit early in the scalar-engine FIFO.
- PSUM: backbone pool 6 banks; mu/ls conv banks are reused for the
  ws/wm sums (disjoint lifetimes) to stay within the 8-bank budget.
"""

import math

import ml_dtypes
import numpy as np

import concourse.dve_ops as dve_ops
import concourse.dve_uop as dve_uop
import concourse.mybir as mybir
import concourse.tile as tile
from concourse import bacc, bass_utils
from concourse.tile import add_dep_helper
from concourse.dve_spec import (
    C0, C1, C2, AluOp, Bin, One, Spec, Src0, Src1, Zero, lower, maxx, minn,
)


def _register_dve_op(name, spec):
    """Append a custom DVE op to the registry at runtime (uop table is
    generated per-NEFF at compile time; sha computed here pins it)."""
    for op in dve_ops.OPS:
        if op.name == name:
            return op
    row = dve_ops._CUSTOM_DVE_ROW_BASE + len(dve_ops.OPS)
    assert row < 0x20
    dve_ops._SUB_OPCODE_FOR_NAME[name] = row
    shas = {}
    for ver in ("v3", "v4"):
        tmp = dve_uop.DveOpSpec(
            name=name, opcode=row, uops=lower(spec, ver=ver),
            rd1_en=dve_ops.has_src1(spec),
        )
        shas[ver] = tmp.sha(ver)
    op = dve_ops.DveOp(name, spec, subdim=False, uops_sha=shas)
    dve_ops.OPS.append(op)
    dve_ops.CUSTOM_DVE_SPECS[name] = spec
    return op


# out = max(in0 + s0, min(in1 - s1, 0)) — fused ELU tail:
# in0 = pre-activation (PSUM), s0 = per-partition bias, in1 = exp(x+b).
ELU_FUSED = _register_dve_op(
    "ELU_FUSED_ANT",
    Spec(
        body=maxx(Src0 + C0, minn(Src1 - C1, Zero)),
        reference=lambda in0, in1, s0, s1, imm2: np.maximum(
            in0 + s0, np.minimum(in1 - s1, 0.0)
        ),
    ),
)
# out = in0 * clip(in1, s0, s1) — weights times clipped inverse variance.
MUL_CLIP = _register_dve_op(
    "MUL_CLIP_ANT",
    Spec(
        body=Src0 * minn(maxx(Src1, C0), C1),
        reference=lambda in0, in1, s0, s1, imm2: in0 * np.minimum(
            np.maximum(in1, s0), s1
        ),
    ),
)



# Problem shape constants (fixed by the task).
B = 32768
IN_DIM = 512
IN_DIM_NG = 480
H1, H2 = 512, 256
E, EH = 8, 256
A = 12                    # ACT_DIM
G = E * A                 # 96 rows: (expert, action) pairs
NCORES = 8
BL = B // NCORES          # 4096 batch rows per core
NB = 512                  # batch columns per tile
NT = BL // NB             # batch tiles per core
OUT_ROWS = 2 * A + E      # mu(12) + sigma(12) + weights(8)

F32 = mybir.dt.float32
BF16 = mybir.dt.bfloat16
AF = mybir.ActivationFunctionType
OP = mybir.AluOpType

IVAR_LO = math.exp(-4.0)  # clip(log_std, -5, 2) in exp(-2*ls) domain
IVAR_HI = math.exp(10.0)

# weight matrices in k-interleaved [128, KT*M] layout: (name, KT, M)
WSPECS = {
    "gW1": (4, H1), "gW2": (4, H2), "gW3x": (2, G),
    "eW1p": (4, H1), "eW2": (4, H2), "hW": (2, E * EH),
    "Wmu": (16, G), "Wls": (16, G),
}
# bias pack columns: name -> (col, ncols, rows)
BCOLS = {}
_c = 0
for _nm, _n, _p in [("gb1", 4, 128), ("gb2", 2, 128), ("gb3h", 1, 96),
                    ("eb2", 2, 128), ("hb", 16, 128), ("cbmu", 1, 96),
                    ("cbls2", 1, 96)]:
    BCOLS[_nm] = (_c, _n, _p)
    _c += _n
NBIAS = _c

_CACHE: dict = {}


def _build_nc():
    nc = bacc.Bacc(
        "TRN2", target_bir_lowering=False, debug=False, num_devices=NCORES
    )

    def din(name, shape, dt):
        return nc.dram_tensor(name, list(shape), dt, kind="ExternalInput").ap()

    xT = din("xT", (128, 4 * BL), BF16)
    xngT = din("xngT", (128, 4 * BL), BF16)
    wd = {n: din(n, (128, kt * m), BF16) for n, (kt, m) in WSPECS.items()}
    bias_d = din("bias", (128, NBIAS), F32)
    Ssum_d = din("Ssum", (G, A), BF16)
    out = nc.dram_tensor("out", [OUT_ROWS, BL], F32, kind="ExternalOutput").ap()

    with tile.TileContext(nc) as tc:
        with (
            tc.tile_pool(name="wpool", bufs=1) as wpool,
            tc.tile_pool(name="apool", bufs=2) as apool,
            tc.tile_pool(name="psum", bufs=1, space="PSUM") as psum,
        ):
            bias_sb = wpool.tile([128, NBIAS], F32, name="bias", tag="bias",
                                 bufs=1)
            nc.gpsimd.dma_start(bias_sb[:], bias_d[:])
            Ssum_t = wpool.tile([G, A], BF16, name="Ssum", tag="Ssum", bufs=1)
            nc.gpsimd.dma_start(Ssum_t[:], Ssum_d[:])
            ws = {}
            for name, (kt, m) in WSPECS.items():
                t = wpool.tile([128, kt * m], BF16, name=name, tag=name,
                               bufs=1)
                if name in ("gW1", "eW1p"):
                    for kk in range(kt):
                        nc.gpsimd.dma_start(
                            t[:, kk * m:(kk + 1) * m],
                            wd[name][:, kk * m:(kk + 1) * m],
                        )
                else:
                    nc.gpsimd.dma_start(t[:], wd[name][:])
                ws[name] = t
            wb_all = wpool.tile([G, BL], F32, name="wb_all", tag="wb_all",
                                bufs=1)
            vt_all = wpool.tile([A, BL], F32, name="vt_all", tag="vt_all",
                                bufs=1)
            sig_all = wpool.tile([A, BL], F32, name="sig_all", tag="sig_all",
                                 bufs=1)

            def wsl(name, t, m0, mm):
                kt, m = WSPECS[name]
                return ws[name][:, t * m + m0:t * m + m0 + mm]

            def bcol(name, i, rows=128):
                c0 = BCOLS[name][0]
                return bias_sb[:rows, c0 + i:c0 + i + 1]

            def linear_elu(n, ins, wname, bname, Mtot, name, mrange=None):
                """ins: list of rhs k-tile APs. bname None => bias already
                folded into the matmul (ones-row trick)."""
                outs = []
                for mi, m0 in enumerate(range(0, Mtot, 128)):
                    if mrange is not None and mi not in mrange:
                        continue
                    mm = min(128, Mtot - m0)
                    ps = psum.tile([mm, NB], F32, name=f"{name}ps{mi}_{n}",
                                   tag="bb", bufs=6)
                    for ki, rhs in enumerate(ins):
                        nc.tensor.matmul(
                            ps[:], wsl(wname, ki, m0, mm), rhs,
                            start=(ki == 0), stop=(ki == len(ins) - 1),
                        )
                    ex = apool.tile([mm, NB], BF16, name=f"{name}e{mi}_{n}",
                                    tag="elu_e", bufs=8)
                    if bname is None:
                        nc.scalar.activation(ex[:], ps[:], AF.Exp)
                    else:
                        nc.scalar.activation(ex[:], ps[:], AF.Exp,
                                             bias=bcol(bname, mi))
                    ot = apool.tile([mm, NB], BF16, name=f"{name}o{mi}_{n}",
                                    tag=f"{name}o{mi}", bufs=2)
                    b_ap = 0.0 if bname is None else bcol(bname, mi)
                    nc.vector._custom_dve(
                        ELU_FUSED, out=ot[:], in0=ps[:], in1=ex[:],
                        s0=b_ap, s1=1.0,
                    )
                    outs.append(ot[:])
                return outs

            def conv_mm(n, eh, wname, tag):
                ps = psum.tile([G, NB], F32, name=f"{tag}ps_{n}", tag=tag,
                               bufs=1)
                for ki in range(16):
                    nc.tensor.matmul(
                        ps[:], wsl(wname, ki, 0, G), eh[ki],
                        start=(ki == 0), stop=(ki == 15),
                    )
                return ps

            def epi_mid(n, ls_ps, mu_ps, wb):
                """inv-var + weighted terms for tile n (inputs all ready)."""
                ivr = apool.tile([G, NB], BF16, name=f"ivr_{n}", tag="ivr",
                                 bufs=2)
                ivr_i = nc.scalar.activation(ivr[:], ls_ps[:], AF.Exp,
                                             bias=bcol("cbls2", 0, G),
                                             scale=-2.0)
                t1 = apool.tile([G, NB], BF16, name=f"t1_{n}", tag="t1",
                                bufs=2)
                nc.vector._custom_dve(
                    MUL_CLIP, out=t1[:], in0=wb, in1=ivr[:],
                    s0=IVAR_LO, s1=IVAR_HI,
                )
                # t2 = (mu + cb_mu) * t1 in one pass.
                t2 = apool.tile([G, NB], BF16, name=f"t2_{n}", tag="t2",
                                bufs=2)
                nc.vector.scalar_tensor_tensor(
                    t2[:], mu_ps[:], bcol("cbmu", 0, G), t1[:],
                    OP.add, OP.mult,
                )
                return t1, t2, ivr_i

            def epi_final(n, t1, t2):
                n0 = n * NB
                ws_ps = psum.tile([A, NB], F32, name=f"wsps_{n}", tag="mu",
                                  bufs=1)
                nc.tensor.matmul(ws_ps[:], Ssum_t[:], t1[:])
                wm_ps = psum.tile([A, NB], F32, name=f"wmps_{n}", tag="ls",
                                  bufs=1)
                nc.tensor.matmul(wm_ps[:], Ssum_t[:], t2[:])

                vt = vt_all[:, n0:n0 + NB]
                nc.vector.reciprocal_approx_fast(out=vt, in_=ws_ps[:])
                mt = apool.tile([A, NB], F32, name=f"mt_{n}", tag="mt",
                                bufs=2)
                nc.vector.tensor_mul(mt[:], vt, wm_ps[:])

                nc.sync.dma_start(out[0:A, n0:n0 + NB], mt[:])
                for ei in range(E):
                    nc.sync.dma_start(
                        out[2 * A + ei:2 * A + ei + 1, n0:n0 + NB],
                        wb_all[A * ei:A * ei + 1, n0:n0 + NB],
                    )

            # Software pipeline: tile n's layer-transition funnels (g2->w96,
            # h2->eh) are covered by tile n-1's 16-matmul conv chunks, which
            # sit AFTER the funnel producers in the PE FIFO.
            pending = None
            for n in range(NT):
                n0 = n * NB

                x_sb = apool.tile([128, 4 * NB], BF16, name=f"x_{n}",
                                  tag="x", bufs=3)
                xng_sb = apool.tile([128, 4 * NB], BF16, name=f"xng_{n}",
                                    tag="xng", bufs=3)
                xTr = xT.rearrange("p (t b) -> p t b", t=4)
                xngTr = xngT.rearrange("p (t b) -> p t b", t=4)
                for t in range(4):
                    nc.sync.dma_start(
                        x_sb[:, t * NB:(t + 1) * NB],
                        xTr[:, t, n0:n0 + NB],
                    )
                    nc.sync.dma_start(
                        xng_sb[:, t * NB:(t + 1) * NB],
                        xngTr[:, t, n0:n0 + NB],
                    )
                x_in = [x_sb[:, t * NB:(t + 1) * NB] for t in range(4)]
                xng_in = [xng_sb[:, t * NB:(t + 1) * NB] for t in range(4)]

                # Gating network (first tile: h1 halves fill the g1->g2
                # and g2->w96 funnels; later tiles use prev-tile conv MMs).
                g1 = linear_elu(n, x_in, "gW1", "gb1", H1, "g1")
                if pending is None:
                    h1a = linear_elu(n, xng_in, "eW1p", None, H1, "h1",
                                     mrange=(0, 1))
                g2 = linear_elu(n, g1, "gW2", "gb2", H2, "g2")
                if pending is not None:
                    pn, peh, pwb = pending
                    p_mu = conv_mm(pn, peh, "Wmu", "mu")
                else:
                    h1 = h1a + linear_elu(n, xng_in, "eW1p", None, H1, "h1",
                                          mrange=(2, 3))
                w96_ps = psum.tile([G, NB], F32, name=f"w96ps_{n}", tag="bb",
                                   bufs=6)
                for ki in range(2):
                    nc.tensor.matmul(
                        w96_ps[:], wsl("gW3x", ki, 0, G), g2[ki],
                        start=(ki == 0), stop=(ki == 1),
                    )

                # Expert backbone + hidden (eb1 folded via ones-row pad).
                if pending is not None:
                    h1 = linear_elu(n, xng_in, "eW1p", None, H1, "h1")
                    p_ls = conv_mm(pn, peh, "Wls", "ls")
                h2 = linear_elu(n, h1, "eW2", "eb2", H2, "h2")

                # ACT-queue tail work: emitted after the backbone exps so the
                # funnel-critical exps run first on the scalar engine.
                # sigmoid(z) = 0.5*tanh(0.5 z) + 0.5 (tanh is in Exp's set).
                th = apool.tile([G, NB], F32, name=f"th_{n}", tag="th",
                                bufs=2)
                nc.scalar.activation(th[:], w96_ps[:], AF.Tanh,
                                     bias=bcol("gb3h", 0, G), scale=0.5)
                wb = wb_all[:, n0:n0 + NB]
                nc.vector.tensor_scalar(wb, th[:], 0.5, 0.5, OP.mult, OP.add)
                if pending is not None:
                    pt1, pt2, _ = epi_mid(pn, p_ls, p_mu, pwb)

                eh = linear_elu(n, h2, "hW", "hb", E * EH, "eh")
                if pending is not None:
                    epi_final(pn, pt1, pt2)
                pending = (n, eh, wb)

            pn, peh, pwb = pending
            p_mu = conv_mm(pn, peh, "Wmu", "mu")
            p_ls = conv_mm(pn, peh, "Wls", "ls")
            pt1, pt2, ivr_i = epi_mid(pn, p_ls, p_mu, pwb)

            # Deferred sigma pass, split so the big Sqrt overlaps the last
            # tile's sums/combine instead of running serially at the end.
            # The explicit dep pins it AFTER the final Exp on the scalar
            # engine (else the scheduler hoists it mid-kernel and thrashes
            # the activation table, as measured in v2).
            nsm = (NT - 1) * NB
            sq1 = nc.scalar.activation(sig_all[:, :nsm], vt_all[:, :nsm],
                                       AF.Sqrt)
            add_dep_helper(sq1.ins, ivr_i.ins,
                           reason="big sqrt after last exp (table order)")
            nc.sync.dma_start(out[A:2 * A, 0:nsm], sig_all[:, :nsm])
            epi_final(pn, pt1, pt2)
            nc.scalar.activation(sig_all[:, nsm:], vt_all[:, nsm:], AF.Sqrt)
            nc.sync.dma_start(out[A:2 * A, nsm:], sig_all[:, nsm:])

    nc.compile()
    return nc


def _interleave_k(w):
    """[K, M] -> [128, (K/128)*M] with column t*M+m = W[t*128+p, m]."""
    K, M = w.shape
    kt = K // 128
    return np.ascontiguousarray(
        w.reshape(kt, 128, M).transpose(1, 0, 2).reshape(128, kt * M)
    )


def _prep_shared(inputs):
    bf = ml_dtypes.bfloat16
    f32 = np.float32

    def a32(x):
        return np.asarray(x, f32)

    cW = a32(inputs["cW"])      # [E, 2A, EH]
    cb = a32(inputs["cb"])      # [E, 2A]
    Wmu = np.zeros((E * EH, G), f32)
    Wls = np.zeros((E * EH, G), f32)
    for e in range(E):
        Wmu[EH * e:EH * (e + 1), A * e:A * (e + 1)] = cW[e, :A, :].T
        Wls[EH * e:EH * (e + 1), A * e:A * (e + 1)] = cW[e, A:, :].T
    eW1p = np.zeros((512, H1), f32)
    eW1p[:IN_DIM_NG] = a32(inputs["eW1"])
    eW1p[IN_DIM_NG] = a32(inputs["eb1"])   # bias row; input row 480 is ones

    mats = {
        "gW1": a32(inputs["gW1"]), "gW2": a32(inputs["gW2"]),
        "gW3x": np.repeat(a32(inputs["gW3"]), A, axis=1),
        "eW1p": eW1p, "eW2": a32(inputs["eW2"]), "hW": a32(inputs["hW"]),
        "Wmu": Wmu, "Wls": Wls,
    }
    shared = {n: _interleave_k(m).astype(bf) for n, m in mats.items()}

    bias = np.zeros((128, NBIAS), f32)

    def put(name, vec):
        c0, ncols, rows = BCOLS[name]
        v = np.asarray(vec, f32).reshape(-1)
        for i in range(ncols):
            seg = v[128 * i:128 * i + rows]
            bias[:len(seg), c0 + i] = seg

    put("gb1", inputs["gb1"])
    put("gb2", inputs["gb2"])
    put("gb3h", 0.5 * np.repeat(a32(inputs["gb3"]), A))
    put("eb2", inputs["eb2"])
    put("hb", inputs["hb"])
    put("cbmu", cb[:, :A].reshape(-1))
    put("cbls2", -2.0 * cb[:, A:].reshape(-1))
    shared["bias"] = bias
    shared["Ssum"] = np.tile(np.eye(A, dtype=f32), (E, 1)).astype(bf)
    return shared


def get_nc():
    if "nc" not in _CACHE:
        _CACHE["nc"] = _build_nc()
    return _CACHE["nc"]


def make_in_maps(inputs):
    bf = ml_dtypes.bfloat16
    f32 = np.float32
    shared = _prep_shared(inputs)
    x = np.asarray(inputs["x"], f32)
    xng_full = np.zeros((B, 512), f32)
    xng_full[:, :IN_DIM_NG] = np.asarray(inputs["x_no_goal"], f32)
    xng_full[:, IN_DIM_NG] = 1.0          # ones feature -> eb1 bias row
    in_maps = []
    for c in range(NCORES):
        sl = slice(c * BL, (c + 1) * BL)
        m = dict(shared)
        m["xT"] = _interleave_k(x[sl].T).astype(bf)
        m["xngT"] = _interleave_k(xng_full[sl].T).astype(bf)
        in_maps.append(m)
    return in_maps


def unshard(results):
    full = np.concatenate(
        [np.asarray(results[c]["out"], np.float32) for c in range(NCORES)],
        axis=1,
    )  # [32, B]
    mu = np.ascontiguousarray(full[0:A].T)
    sigma = np.ascontiguousarray(full[A:2 * A].T)
    wts = np.ascontiguousarray(full[2 * A:].T)
    return mu, sigma, wts


def kernel(**inputs):
    nc = get_nc()
    in_maps = make_in_maps(inputs)
    res = bass_utils.run_bass_kernel_spmd(
        nc, in_maps, core_ids=list(range(NCORES))
    )
    return unshard(res.results)


# revision 30
# speedup vs baseline: 1.0043x; 1.0043x over previous
"""Trainium2 Bass kernel for the ActorMCP mixture-of-experts policy network.

Data-parallel over 8 NeuronCores: batch 32768 -> 4096 rows/core, weights
replicated. All activations live transposed [feature(partitions), batch(free)]
so every layer is out.T = W.T @ x.T with W in its natural [fan_in, fan_out]
layout as the stationary matmul operand.

Engine budget choices (from trace analysis):
- Scalar engine runs ONLY Exp/Tanh (one activation-table set) for the
  whole batch loop; the single Sqrt runs once at the end over all batch
  columns (forced there by its data dependency), so there are exactly
  2 ACT_TABLE_LOADs in the kernel.
- sigmoid(z) = 0.5*tanh(0.5 z) + 0.5 (tanh shares the exp table set).
- ELU = max(x+b, min(exp(x+b)-1, 0)) as ONE fused custom DVE op after the
  ACT exp; w*clip(inv_var) is a second custom op. Custom DVE uop tables
  are registered at runtime and baked into the NEFF.
- GpSimd does no element-wise work (measured 7.5us/op + DVE port stalls);
  it only issues the weight DMAs so they ride a different queue than the
  per-tile input loads on Sync.
- Weights/inputs host-packed k-tile-interleaved [128, K/128*M] so big
  contiguous DMAs feed SBUF; first-consumed slabs split per k-chunk so
  the first matmul starts as early as possible.
- Software pipelining: tile n's per-expert GEMMs + mixture epilogue are
  emitted inside tile n+1's backbone at points chosen so the PE FIFO
  always holds ready work across the narrow layer-transition funnels,
  and the funnel-critical exps sit early in the scalar-engine FIFO.
- PSUM: backbone pool 6 banks; mu/ls conv banks are reused for the
  ws/wm sums (disjoint lifetimes) to stay within the 8-bank budget.
"""

import math

import ml_dtypes
import numpy as np

import concourse.dve_ops as dve_ops
import concourse.dve_uop as dve_uop
import concourse.mybir as mybir
import concourse.tile as tile
from concourse import bacc, bass_utils
from concourse.tile import add_dep_helper
from concourse.dve_spec import (
    C0, C1, C2, AluOp, Bin, One, Spec, Src0, Src1, Zero, lower, maxx, minn,
)


def _register_dve_op(name, spec):
    """Append a custom DVE op to the registry at runtime (uop table is
    generated per-NEFF at compile time; sha computed here pins it)."""
    for op in dve_ops.OPS:
        if op.name == name:
            return op
    row = dve_ops._CUSTOM_DVE_ROW_BASE + len(dve_ops.OPS)
    assert row < 0x20
    dve_ops._SUB_OPCODE_FOR_NAME[name] = row
    shas = {}
    for ver in ("v3", "v4"):
        tmp = dve_uop.DveOpSpec(
            name=name, opcode=row, uops=lower(spec, ver=ver),
            rd1_en=dve_ops.has_src1(spec),
        )
        shas[ver] = tmp.sha(ver)
    op = dve_ops.DveOp(name, spec, subdim=False, uops_sha=shas)
    dve_ops.OPS.append(op)
    dve_ops.CUSTOM_DVE_SPECS[name] = spec
    return op


# out = max(in0 + s0, min(in1 - s1, 0)) — fused ELU tail:
# in0 = pre-activation (PSUM), s0 = per-partition bias, in1 = exp(x+b).
ELU_FUSED = _register_dve_op(
    "ELU_FUSED_ANT",
    Spec(
        body=maxx(Src0 + C0, minn(Src1 - C1, Zero)),
        reference=lambda in0, in1, s0, s1, imm2: np.maximum(
            in0 + s0, np.minimum(in1 - s1, 0.0)
        ),
    ),
)
# out = in0 * clip(in1, s0, s1) — weights times clipped inverse variance.
MUL_CLIP = _register_dve_op(
    "MUL_CLIP_ANT",
    Spec(
        body=Src0 * minn(maxx(Src1, C0), C1),
        reference=lambda in0, in1, s0, s1, imm2: in0 * np.minimum(
            np.maximum(in1, s0), s1
        ),
    ),
)



# Problem shape constants (fixed by the task).
B = 32768
IN_DIM = 512
IN_DIM_NG = 480
H1, H2 = 512, 256
E, EH = 8, 256
A = 12                    # ACT_DIM
G = E * A                 # 96 rows: (expert, action) pairs
NCORES = 8
BL = B // NCORES          # 4096 batch rows per core
NB = 512                  # batch columns per tile
NT = BL // NB             # batch tiles per core
OUT_ROWS = 2 * A + E      # mu(12) + sigma(12) + weights(8)

F32 = mybir.dt.float32
BF16 = mybir.dt.bfloat16
AF = mybir.ActivationFunctionType
OP = mybir.AluOpType

IVAR_LO = math.exp(-4.0)  # clip(log_std, -5, 2) in exp(-2*ls) domain
IVAR_HI = math.exp(10.0)

# weight matrices in k-interleaved [128, KT*M] layout: (name, KT, M)
WSPECS = {
    "gW1": (4, H1), "eW1p": (4, H1), "gW2": (4, H2), "gW3x": (2, G),
    "eW2": (4, H2), "hW": (2, E * EH),
    "Wmu": (16, G), "Wls": (16, G),
}
# bias pack columns: name -> (col, ncols, rows)
BCOLS = {}
_c = 0
for _nm, _n, _p in [("gb1", 4, 128), ("gb2", 2, 128), ("gb3h", 1, 96),
                    ("eb2", 2, 128), ("hb", 16, 128), ("cbmu", 1, 96),
                    ("cbls2", 1, 96)]:
    BCOLS[_nm] = (_c, _n, _p)
    _c += _n
NBIAS = _c

_CACHE: dict = {}


def _build_nc():
    nc = bacc.Bacc(
        "TRN2", target_bir_lowering=False, debug=False, num_devices=NCORES
    )

    def din(name, shape, dt):
        return nc.dram_tensor(name, list(shape), dt, kind="ExternalInput").ap()

    xT = din("xT", (128, 4 * BL), BF16)
    xngT = din("xngT", (128, 4 * BL), BF16)
    wd = {n: din(n, (128, kt * m), BF16) for n, (kt, m) in WSPECS.items()}
    bias_d = din("bias", (128, NBIAS), F32)
    Ssum_d = din("Ssum", (G, A), BF16)
    out = nc.dram_tensor("out", [OUT_ROWS, BL], F32, kind="ExternalOutput").ap()

    with tile.TileContext(nc) as tc:
        with (
            tc.tile_pool(name="wpool", bufs=1) as wpool,
            tc.tile_pool(name="apool", bufs=2) as apool,
            tc.tile_pool(name="psum", bufs=1, space="PSUM") as psum,
        ):
            bias_sb = wpool.tile([128, NBIAS], F32, name="bias", tag="bias",
                                 bufs=1)
            nc.scalar.dma_start(bias_sb[:], bias_d[:])
            Ssum_t = wpool.tile([G, A], BF16, name="Ssum", tag="Ssum", bufs=1)
            nc.scalar.dma_start(Ssum_t[:], Ssum_d[:])
            ws = {}
            for name, (kt, m) in WSPECS.items():
                t = wpool.tile([128, kt * m], BF16, name=name, tag=name,
                               bufs=1)
                if name in ("gW1", "eW1p"):
                    for kk in range(kt):
                        nc.gpsimd.dma_start(
                            t[:, kk * m:(kk + 1) * m],
                            wd[name][:, kk * m:(kk + 1) * m],
                        )
                else:
                    nc.gpsimd.dma_start(t[:], wd[name][:])
                ws[name] = t
            wb_all = wpool.tile([G, BL], F32, name="wb_all", tag="wb_all",
                                bufs=1)
            vt_all = wpool.tile([A, BL], F32, name="vt_all", tag="vt_all",
                                bufs=1)
            sig_all = wpool.tile([A, BL], F32, name="sig_all", tag="sig_all",
                                 bufs=1)

            def wsl(name, t, m0, mm):
                kt, m = WSPECS[name]
                return ws[name][:, t * m + m0:t * m + m0 + mm]

            def bcol(name, i, rows=128):
                c0 = BCOLS[name][0]
                return bias_sb[:rows, c0 + i:c0 + i + 1]

            def linear_elu(n, ins, wname, bname, Mtot, name, mrange=None):
                """ins: list of rhs k-tile APs. bname None => bias already
                folded into the matmul (ones-row trick)."""
                outs = []
                for mi, m0 in enumerate(range(0, Mtot, 128)):
                    if mrange is not None and mi not in mrange:
                        continue
                    mm = min(128, Mtot - m0)
                    ps = psum.tile([mm, NB], F32, name=f"{name}ps{mi}_{n}",
                                   tag="bb", bufs=6)
                    for ki, rhs in enumerate(ins):
                        nc.tensor.matmul(
                            ps[:], wsl(wname, ki, m0, mm), rhs,
                            start=(ki == 0), stop=(ki == len(ins) - 1),
                        )
                    ex = apool.tile([mm, NB], BF16, name=f"{name}e{mi}_{n}",
                                    tag="elu_e", bufs=8)
                    if bname is None:
                        nc.scalar.activation(ex[:], ps[:], AF.Exp)
                    else:
                        nc.scalar.activation(ex[:], ps[:], AF.Exp,
                                             bias=bcol(bname, mi))
                    ot = apool.tile([mm, NB], BF16, name=f"{name}o{mi}_{n}",
                                    tag=f"{name}o{mi}", bufs=2)
                    b_ap = 0.0 if bname is None else bcol(bname, mi)
                    nc.vector._custom_dve(
                        ELU_FUSED, out=ot[:], in0=ps[:], in1=ex[:],
                        s0=b_ap, s1=1.0,
                    )
                    outs.append(ot[:])
                return outs

            def conv_mm(n, eh, wname, tag):
                ps = psum.tile([G, NB], F32, name=f"{tag}ps_{n}", tag=tag,
                               bufs=1)
                for ki in range(16):
                    nc.tensor.matmul(
                        ps[:], wsl(wname, ki, 0, G), eh[ki],
                        start=(ki == 0), stop=(ki == 15),
                    )
                return ps

            def epi_mid(n, ls_ps, mu_ps, wb):
                """inv-var + weighted terms for tile n (inputs all ready)."""
                ivr = apool.tile([G, NB], BF16, name=f"ivr_{n}", tag="ivr",
                                 bufs=2)
                ivr_i = nc.scalar.activation(ivr[:], ls_ps[:], AF.Exp,
                                             bias=bcol("cbls2", 0, G),
                                             scale=-2.0)
                t1 = apool.tile([G, NB], BF16, name=f"t1_{n}", tag="t1",
                                bufs=2)
                nc.vector._custom_dve(
                    MUL_CLIP, out=t1[:], in0=wb, in1=ivr[:],
                    s0=IVAR_LO, s1=IVAR_HI,
                )
                # t2 = (mu + cb_mu) * t1 in one pass.
                t2 = apool.tile([G, NB], BF16, name=f"t2_{n}", tag="t2",
                                bufs=2)
                nc.vector.scalar_tensor_tensor(
                    t2[:], mu_ps[:], bcol("cbmu", 0, G), t1[:],
                    OP.add, OP.mult,
                )
                return t1, t2, ivr_i

            def epi_final(n, t1, t2):
                n0 = n * NB
                ws_ps = psum.tile([A, NB], F32, name=f"wsps_{n}", tag="mu",
                                  bufs=1)
                nc.tensor.matmul(ws_ps[:], Ssum_t[:], t1[:])
                wm_ps = psum.tile([A, NB], F32, name=f"wmps_{n}", tag="ls",
                                  bufs=1)
                nc.tensor.matmul(wm_ps[:], Ssum_t[:], t2[:])

                vt = vt_all[:, n0:n0 + NB]
                nc.vector.reciprocal_approx_fast(out=vt, in_=ws_ps[:])
                mt = apool.tile([A, NB], F32, name=f"mt_{n}", tag="mt",
                                bufs=2)
                nc.vector.tensor_mul(mt[:], vt, wm_ps[:])

                nc.sync.dma_start(out[0:A, n0:n0 + NB], mt[:])
                for ei in range(E):
                    nc.sync.dma_start(
                        out[2 * A + ei:2 * A + ei + 1, n0:n0 + NB],
                        wb_all[A * ei:A * ei + 1, n0:n0 + NB],
                    )

            # Software pipeline: tile n's layer-transition funnels (g2->w96,
            # h2->eh) are covered by tile n-1's 16-matmul conv chunks, which
            # sit AFTER the funnel producers in the PE FIFO.
            pending = None
            for n in range(NT):
                n0 = n * NB

                x_sb = apool.tile([128, 4 * NB], BF16, name=f"x_{n}",
                                  tag="x", bufs=3)
                xng_sb = apool.tile([128, 4 * NB], BF16, name=f"xng_{n}",
                                    tag="xng", bufs=3)
                xTr = xT.rearrange("p (t b) -> p t b", t=4)
                xngTr = xngT.rearrange("p (t b) -> p t b", t=4)
                for t in range(4):
                    nc.sync.dma_start(
                        x_sb[:, t * NB:(t + 1) * NB],
                        xTr[:, t, n0:n0 + NB],
                    )
                    nc.sync.dma_start(
                        xng_sb[:, t * NB:(t + 1) * NB],
                        xngTr[:, t, n0:n0 + NB],
                    )
                x_in = [x_sb[:, t * NB:(t + 1) * NB] for t in range(4)]
                xng_in = [xng_sb[:, t * NB:(t + 1) * NB] for t in range(4)]

                # Gating network (first tile: h1 halves fill the g1->g2
                # and g2->w96 funnels; later tiles use prev-tile conv MMs).
                g1 = linear_elu(n, x_in, "gW1", "gb1", H1, "g1")
                if pending is None:
                    h1a = linear_elu(n, xng_in, "eW1p", None, H1, "h1",
                                     mrange=(0, 1))
                g2 = linear_elu(n, g1, "gW2", "gb2", H2, "g2")
                if pending is not None:
                    pn, peh, pwb = pending
                    p_mu = conv_mm(pn, peh, "Wmu", "mu")
                else:
                    h1 = h1a + linear_elu(n, xng_in, "eW1p", None, H1, "h1",
                                          mrange=(2, 3))
                w96_ps = psum.tile([G, NB], F32, name=f"w96ps_{n}", tag="bb",
                                   bufs=6)
                for ki in range(2):
                    nc.tensor.matmul(
                        w96_ps[:], wsl("gW3x", ki, 0, G), g2[ki],
                        start=(ki == 0), stop=(ki == 1),
                    )

                # Expert backbone + hidden (eb1 folded via ones-row pad).
                if pending is not None:
                    h1 = linear_elu(n, xng_in, "eW1p", None, H1, "h1")
                    p_ls = conv_mm(pn, peh, "Wls", "ls")
                h2 = linear_elu(n, h1, "eW2", "eb2", H2, "h2")

                # ACT-queue tail work: emitted after the backbone exps so the
                # funnel-critical exps run first on the scalar engine.
                # sigmoid(z) = 0.5*tanh(0.5 z) + 0.5 (tanh is in Exp's set).
                th = apool.tile([G, NB], F32, name=f"th_{n}", tag="th",
                                bufs=2)
                nc.scalar.activation(th[:], w96_ps[:], AF.Tanh,
                                     bias=bcol("gb3h", 0, G), scale=0.5)
                wb = wb_all[:, n0:n0 + NB]
                nc.vector.tensor_scalar(wb, th[:], 0.5, 0.5, OP.mult, OP.add)
                if pending is not None:
                    pt1, pt2, _ = epi_mid(pn, p_ls, p_mu, pwb)

                eh = linear_elu(n, h2, "hW", "hb", E * EH, "eh")
                if pending is not None:
                    epi_final(pn, pt1, pt2)
                pending = (n, eh, wb)

            pn, peh, pwb = pending
            p_mu = conv_mm(pn, peh, "Wmu", "mu")
            p_ls = conv_mm(pn, peh, "Wls", "ls")
            pt1, pt2, ivr_i = epi_mid(pn, p_ls, p_mu, pwb)

            # Deferred sigma pass, split so the big Sqrt overlaps the last
            # tile's sums/combine instead of running serially at the end.
            # The explicit dep pins it AFTER the final Exp on the scalar
            # engine (else the scheduler hoists it mid-kernel and thrashes
            # the activation table, as measured in v2).
            nsm = (NT - 1) * NB
            sq1 = nc.scalar.activation(sig_all[:, :nsm], vt_all[:, :nsm],
                                       AF.Sqrt)
            add_dep_helper(sq1.ins, ivr_i.ins,
                           reason="big sqrt after last exp (table order)")
            nc.sync.dma_start(out[A:2 * A, 0:nsm], sig_all[:, :nsm])
            epi_final(pn, pt1, pt2)
            nc.scalar.activation(sig_all[:, nsm:], vt_all[:, nsm:], AF.Sqrt)
            nc.sync.dma_start(out[A:2 * A, nsm:], sig_all[:, nsm:])

    nc.compile()
    return nc


def _interleave_k(w):
    """[K, M] -> [128, (K/128)*M] with column t*M+m = W[t*128+p, m]."""
    K, M = w.shape
    kt = K // 128
    return np.ascontiguousarray(
        w.reshape(kt, 128, M).transpose(1, 0, 2).reshape(128, kt * M)
    )


def _prep_shared(inputs):
    bf = ml_dtypes.bfloat16
    f32 = np.float32

    def a32(x):
        return np.asarray(x, f32)

    cW = a32(inputs["cW"])      # [E, 2A, EH]
    cb = a32(inputs["cb"])      # [E, 2A]
    Wmu = np.zeros((E * EH, G), f32)
    Wls = np.zeros((E * EH, G), f32)
    for e in range(E):
        Wmu[EH * e:EH * (e + 1), A * e:A * (e + 1)] = cW[e, :A, :].T
        Wls[EH * e:EH * (e + 1), A * e:A * (e + 1)] = cW[e, A:, :].T
    eW1p = np.zeros((512, H1), f32)
    eW1p[:IN_DIM_NG] = a32(inputs["eW1"])
    eW1p[IN_DIM_NG] = a32(inputs["eb1"])   # bias row; input row 480 is ones

    mats = {
        "gW1": a32(inputs["gW1"]), "gW2": a32(inputs["gW2"]),
        "gW3x": np.repeat(a32(inputs["gW3"]), A, axis=1),
        "eW1p": eW1p, "eW2": a32(inputs["eW2"]), "hW": a32(inputs["hW"]),
        "Wmu": Wmu, "Wls": Wls,
    }
    shared = {n: _interleave_k(m).astype(bf) for n, m in mats.items()}

    bias = np.zeros((128, NBIAS), f32)

    def put(name, vec):
        c0, ncols, rows = BCOLS[name]
        v = np.asarray(vec, f32).reshape(-1)
        for i in range(ncols):
            seg = v[128 * i:128 * i + rows]
            bias[:len(seg), c0 + i] = seg

    put("gb1", inputs["gb1"])
    put("gb2", inputs["gb2"])
    put("gb3h", 0.5 * np.repeat(a32(inputs["gb3"]), A))
    put("eb2", inputs["eb2"])
    put("hb", inputs["hb"])
    put("cbmu", cb[:, :A].reshape(-1))
    put("cbls2", -2.0 * cb[:, A:].reshape(-1))
    shared["bias"] = bias
    shared["Ssum"] = np.tile(np.eye(A, dtype=f32), (E, 1)).astype(bf)
    return shared


def get_nc():
    if "nc" not in _CACHE:
        _CACHE["nc"] = _build_nc()
    return _CACHE["nc"]


def make_in_maps(inputs):
    bf = ml_dtypes.bfloat16
    f32 = np.float32
    shared = _prep_shared(inputs)
    x = np.asarray(inputs["x"], f32)
    xng_full = np.zeros((B, 512), f32)
    xng_full[:, :IN_DIM_NG] = np.asarray(inputs["x_no_goal"], f32)
    xng_full[:, IN_DIM_NG] = 1.0          # ones feature -> eb1 bias row
    in_maps = []
    for c in range(NCORES):
        sl = slice(c * BL, (c + 1) * BL)
        m = dict(shared)
        m["xT"] = _interleave_k(x[sl].T).astype(bf)
        m["xngT"] = _interleave_k(xng_full[sl].T).astype(bf)
        in_maps.append(m)
    return in_maps


def unshard(results):
    full = np.concatenate(
        [np.asarray(results[c]["out"], np.float32) for c in range(NCORES)],
        axis=1,
    )  # [32, B]
    mu = np.ascontiguousarray(full[0:A].T)
    sigma = np.ascontiguousarray(full[A:2 * A].T)
    wts = np.ascontiguousarray(full[2 * A:].T)
    return mu, sigma, wts


def kernel(**inputs):
    nc = get_nc()
    in_maps = make_in_maps(inputs)
    res = bass_utils.run_bass_kernel_spmd(
        nc, in_maps, core_ids=list(range(NCORES))
    )
    return unshard(res.results)
